# revision 4
# baseline (speedup 1.0000x reference)
"""Trainium2 Bass kernel for the channel-interaction-attention module.

Reference computation (x: (4, 1024, 64, 64) fp32, F = x.ravel()):
    A  = F.view(16384, 1024)          # x.reshape(-1, C)
    Bm = F.view(1024, 16384)          # x.reshape(C, -1)
    S  = Bm @ A                       # (C, C)
    E  = softmax(S, axis=-1)
    U  = E @ Bm                       # (C, N)
    Y  = softmax(U, axis=-1)          # softmax over N = 16384
    out = x + softmax(Y.view(4,1024,64,64), axis=-1)   # softmax over W=64
"""

import numpy as np
import ml_dtypes

import concourse.bass as bass
import concourse.bacc as bacc
import concourse.tile as tile
import concourse.mybir as mybir
from concourse import bass_utils

N_CORES = 8
B, C, H, W = 4, 1024, 64, 64
N = B * H * W            # 16384
NS = N // N_CORES        # 2048 per-core shard
P = 128
MT = C // P              # 8 row-blocks of S / U
KT1 = NS // P            # 16 contraction tiles for GEMM1
KT2 = C // P             # 8 contraction tiles for GEMM2
D1 = KT1 // 2            # 8 DoubleRow steps, GEMM1
D2 = KT2 // 2            # 4 DoubleRow steps, GEMM2

FP32 = mybir.dt.float32
BF16 = mybir.dt.bfloat16
FP8 = mybir.dt.float8e4
EXP = mybir.ActivationFunctionType.Exp
DR = mybir.MatmulPerfMode.DoubleRow
AX = mybir.AxisListType.X


def build_module(repeat: int = 1, fp8: bool = True, collectives: bool = True):
    nc = bacc.Bacc("TRN2", target_bir_lowering=False, debug=False,
                   num_devices=N_CORES if collectives else 1)

    def all_reduce(cc_in, cc_out):
        if collectives:
            nc.gpsimd.collective_compute(
                "AllReduce", mybir.AluOpType.add,
                replica_groups=[list(range(N_CORES))],
                ins=[cc_in.opt()], outs=[cc_out.opt()],
            )
        else:
            nc.sync.dma_start(cc_out[:], cc_in[:])

    a_d = nc.dram_tensor("a_in", [NS, C], FP8, kind="ExternalInput")
    bt_d = nc.dram_tensor("bt_in", [NS, C], FP8, kind="ExternalInput")
    b_d = nc.dram_tensor("b_in", [C, NS], FP8, kind="ExternalInput")
    o_d = nc.dram_tensor("o_out", [C, NS], BF16, kind="ExternalOutput")

    with tile.TileContext(nc) as tc:
        with (
            tc.tile_pool(name="lp", bufs=2) as lp,
            tc.tile_pool(name="upool", bufs=2) as upool,
            tc.tile_pool(name="etbfp", bufs=1) as etbfp,
            tc.tile_pool(name="etp", bufs=1) as etp,
            tc.tile_pool(name="ep", bufs=3) as ep,
            tc.tile_pool(name="srp", bufs=3) as srp,
            tc.tile_pool(name="scp", bufs=4) as scp,
            tc.tile_pool(name="zp", bufs=2) as zp,
            tc.tile_pool(name="wst", bufs=4) as wst,
            tc.tile_pool(name="stat", bufs=2) as stat,
            tc.tile_pool(name="cst", bufs=1) as cst,
            tc.tile_pool(name="ps", bufs=2, space="PSUM") as psp,
            tc.tile_pool(name="dram", bufs=1, space="DRAM") as dram,
        ):
            ubias = cst.tile([P, 1], FP32, tag="ubias")
            nc.vector.memset(ubias[:], -1.5)
            for rep in range(repeat):
                # ---- stream in this rep's operands (prev rep overlaps) ----
                a_t = lp.tile([P, KT1, C], FP8, tag="a")
                bt_t = lp.tile([P, KT1, C], FP8, tag="bt")
                b_t = lp.tile([P, KT2, NS], FP8, tag="b")
                nc.sync.dma_start(
                    a_t[:], a_d[:].rearrange("(k p) c -> p k c", p=P))
                nc.sync.dma_start(
                    bt_t[:], bt_d[:].rearrange("(k p) c -> p k c", p=P))
                nc.sync.dma_start(
                    b_t[:], b_d[:].rearrange("(k p) n -> p k n", p=P))

                # ---- GEMM1: partial S/8 row-blocks, AllReduce by halves ----
                s_in = [dram.tile([P, 4, C], FP8, tag=f"ci{rep}_{h}",
                                  name=f"s_in{rep}_{h}") for h in range(2)]
                s_out = [dram.tile([P, 4, C], FP8, tag=f"co{rep}_{h}",
                                   addr_space="Shared",
                                   name=f"s_out{rep}_{h}") for h in range(2)]
                for g in range(4):
                    ps = psp.tile([P, 2 * C], FP32, tag="ps",
                                  name=f"ps1_{rep}_{g}")
                    for mo in range(2):
                        m = 2 * g + mo
                        for k in range(D1):
                            for nn in range(2):
                                nc.tensor.matmul(
                                    ps[:, mo * C + nn * 512:
                                       mo * C + (nn + 1) * 512],
                                    bt_t[:, 2 * k:2 * k + 2,
                                         m * P:(m + 1) * P],
                                    a_t[:, 2 * k:2 * k + 2,
                                        nn * 512:(nn + 1) * 512],
                                    start=(k == 0), stop=(k == D1 - 1),
                                    perf_mode=DR)
                        sc = scp.tile([P, C], FP8, tag="sc",
                                      name=f"sc_{rep}_{m}")
                        nc.vector.tensor_copy(sc[:], ps[:, mo * C:
                                                        (mo + 1) * C])
                        nc.sync.dma_start(s_in[m // 4][:, m % 4, :], sc[:])
                    if g % 2 == 1:
                        all_reduce(s_in[g // 2], s_out[g // 2])

                # ---- per row-block: softmax(S), E^T via DMA-transpose,
                #      GEMM2, exp-evict U ----
                negmax = stat.tile([P, MT], FP32, tag="nm")
                negmax8 = stat.tile([P, MT], FP32, tag="nm8")
                rsum = stat.tile([P, MT], FP32, tag="rs")
                rscale = stat.tile([P, MT], FP32, tag="rsc")
                acc = stat.tile([P, MT], FP32, tag="ac")
                gsum = stat.tile([P, MT], FP32, tag="gs")
                gscale = stat.tile([P, MT], FP32, tag="gsc")
                et_bf = etbfp.tile([P, KT2, C], BF16, tag="etbf")
                et_t = etp.tile([P, KT2, C], FP8, tag="et")
                u_t = upool.tile([P, MT, NS], FP8, tag="u")
                for m in range(MT):
                    sr = srp.tile([P, C], FP8, tag="sr", name=f"sr_{rep}_{m}")
                    nc.sync.dma_start(sr[:], s_out[m // 4][:, m % 4, :])
                    nc.vector.tensor_reduce(
                        negmax[:, m:m + 1], sr[:], axis=AX,
                        op=mybir.AluOpType.max, negate=True)
                    nc.vector.tensor_scalar_mul(
                        negmax8[:, m:m + 1], negmax[:, m:m + 1], 8.0)
                    e_t = ep.tile([P, C], BF16, tag="e", name=f"e_{rep}_{m}")
                    nc.scalar.activation(
                        e_t[:], sr[:], EXP,
                        bias=negmax8[:, m:m + 1], scale=8.0,
                        accum_out=rsum[:, m:m + 1])
                    nc.vector.reciprocal(rscale[:, m:m + 1], rsum[:, m:m + 1])
                    nc.sync.dma_start(et_bf[:, :, m * P:(m + 1) * P], e_t[:],
                                      transpose=True)
                    nc.vector.tensor_copy(et_t[:, :, m * P:(m + 1) * P],
                                          et_bf[:, :, m * P:(m + 1) * P])
                    ps2 = psp.tile([P, NS], FP32, tag="ps",
                                   name=f"ps2_{rep}_{m}")
                    for k in range(D2):
                        for nq in range(4):
                            nc.tensor.matmul(
                                ps2[:, nq * 512:(nq + 1) * 512],
                                et_t[:, 2 * k:2 * k + 2, m * P:(m + 1) * P],
                                b_t[:, 2 * k:2 * k + 2,
                                    nq * 512:(nq + 1) * 512],
                                start=(k == 0), stop=(k == D2 - 1),
                                perf_mode=DR)
                    # u = exp(rscale*Uraw - 1.5); the -1.5 keeps exp within
                    # fp8e4 range and cancels in the softmax normalization
                    nc.scalar.activation(
                        u_t[:, m, :], ps2[:], EXP,
                        bias=ubias[:], scale=rscale[:, m:m + 1],
                        accum_out=acc[:, m:m + 1])

                # ---- N-softmax denominators: one tiny AllReduce ----
                ls_in = dram.tile([P, MT], FP32, tag=f"li{rep}",
                                  name=f"ls_in{rep}")
                ls_out = dram.tile([P, MT], FP32, tag=f"lo{rep}",
                                   addr_space="Shared", name=f"ls_out{rep}")
                nc.sync.dma_start(ls_in[:], acc[:])
                all_reduce(ls_in, ls_out)
                nc.sync.dma_start(gsum[:], ls_out[:])
                nc.vector.reciprocal(gscale[:], gsum[:])

                # ---- z = softmax_W(u * gscale) ; store ----
                for m in range(MT):
                    z = zp.tile([P, NS], BF16, tag="z", name=f"z_{rep}_{m}")
                    nc.scalar.activation(z[:], u_t[:, m, :], EXP,
                                         bias=0.0, scale=gscale[:, m:m + 1])
                    z3 = z[:].rearrange("p (r w) -> p r w", w=W)
                    wsum = wst.tile([P, NS // W], FP32, tag="ws",
                                    name=f"ws_{rep}_{m}")
                    nc.vector.tensor_reduce(wsum[:], z3, axis=AX,
                                            op=mybir.AluOpType.add)
                    wrecip = wst.tile([P, NS // W], FP32, tag="wr",
                                      name=f"wr_{rep}_{m}")
                    nc.vector.reciprocal(wrecip[:], wsum[:])
                    wb = wrecip[:].unsqueeze(2).broadcast_to((P, NS // W, W))
                    nc.gpsimd.tensor_tensor(z3, z3, wb,
                                            op=mybir.AluOpType.mult)
                    nc.sync.dma_start(o_d[m * P:(m + 1) * P, :], z[:])

    nc.compile()
    return nc


_module_cache = {}


def _get_module(repeat: int = 1, fp8: bool = True, collectives: bool = True):
    key = (repeat, fp8, collectives)
    if key not in _module_cache:
        _module_cache[key] = build_module(repeat, fp8, collectives)
    return _module_cache[key]


def make_in_maps(x: np.ndarray, fp8: bool = True):
    in_dt = ml_dtypes.float8_e4m3
    F = np.ascontiguousarray(x, dtype=np.float32).reshape(-1)
    A = F.reshape(N, C)
    Bm = F.reshape(C, N)
    in_maps = []
    for k in range(N_CORES):
        sl = slice(k * NS, (k + 1) * NS)
        b_f32 = np.ascontiguousarray(Bm[:, sl])
        b_lp = b_f32.astype(in_dt)
        bt_lp = np.ascontiguousarray(b_lp.T)
        # pre-scale A by 1/8 so per-core partial sums of S/8 fit fp8e4
        a_lp = (A[sl] * 0.125).astype(in_dt)
        in_maps.append({
            "a_in": a_lp,
            "bt_in": bt_lp,
            "b_in": b_lp,
        })
    return in_maps


def assemble_output(x: np.ndarray, results):
    term = np.concatenate(
        [results[k]["o_out"].astype(np.float32) for k in range(N_CORES)],
        axis=1)
    return (np.asarray(x, dtype=np.float32)
            + term.reshape(B, C, H, W))


def kernel(x: np.ndarray) -> np.ndarray:
    nc = _get_module()
    in_maps = make_in_maps(x)
    res = bass_utils.run_bass_kernel_spmd(
        nc, in_maps, core_ids=list(range(N_CORES)))
    return assemble_output(x, res.results)
